# revision 9
# baseline (speedup 1.0000x reference)
"""Trainium2 Bass/Tile kernel for EnrichedGeometricEmbedding (v2).

Full-input contract: kernel(**inputs) takes the complete tensors, shards the
batch dim across 8 NeuronCores (B=8 -> 1 batch row per core), runs one SPMD
program via run_bass_kernel_spmd, and gathers the full [8, 1024, 32, 384]
output. Memory-bound: the 50 MB/core output write at 360 GB/s (~140 us) sets
the roofline; everything else must hide behind it.

v2 changes vs the 183 us baseline (all aimed at the 39-us prologue and the
ACT-engine budget):
  - RBF in ONE activation pass: Derivative_Erf(sqrt2*(x - c)) =
    2/sqrt(pi)*exp(-2(x-c)^2); the 2/sqrt(pi) is folded into W rows 0..128.
    All ACT functions used (DErf, Abs, Copy) live in the single
    `erf_derivative` table set -> one preamble table load, none in-flight
    (baseline had 5 at 1.3 us each, mostly on the eigen critical path).
  - Smallest-eigenvalue via Newton iteration from x=0 on the characteristic
    cubic (monotone convergence from below for PSD matrices): an all-DVE
    polynomial chain, no Sqrt/Arctan/Sin tables, no DVE<->ACT sem ping-pong.
    6 iterations give max |dcurv| ~2e-3 (numpy-validated on the reference
    input distribution; out rel-err contribution ~2e-4).
  - Covariance from raw moments (sum nn^T - K m m^T) - no centering pass,
    products can start as soon as neighbors land.
  - kd-major transpose layout: ONE [128,96] PE transpose per 128-group tile
    (baseline: three [128,32] per tile), xdkT row = 3k+d.
  - ebig (2.29 MB, 6.4 us of DMA) replaced by x3s: a quadrant-aligned repack
    [32*(k//8)+d, (k%8)*1024+g] built with 12 tiny stride-3-partition SBUF
    DMAs; per-k broadcast matmul uses lhsT=e3s[32q:32q+3] (tiny constant) and
    rhs=x3s[32q:32q+3, ...], both at walrus-legal partition bases.
  - blob2 is gone; whi moved into the single constant blob.

Steady state (unchanged from baseline): per (k, half) one 512-point tile:
PE broadcast matmul -> DErf -> 4x(whi K=128 + wlo K=6) accumulating f32r
matmuls -> PSUM->SBUF copies alternating ACT/DVE -> one HWDGE DMA scattering
[128, 4, 384] rows to DRAM at 1536 B/row.
"""

import math

import numpy as np

B, S, K, D = 8, 1024, 32, 3
F = 43                      # FEAT_DIM
OUT = 384
G = S                       # groups per core
P = S * K                   # points per core (32768)
NT = G // 128               # group tiles (8)
TOTAL = F * D + 1 + D       # 133

_prog_cache = {}


def _build_program():
    import concourse.bacc as bacc
    import concourse.mybir as mybir
    from concourse.tile import TileContext

    DT = mybir.dt.float32
    DTR = mybir.dt.float32r
    Act = mybir.ActivationFunctionType
    Op = mybir.AluOpType
    X = mybir.AxisListType.X
    SQ2 = math.sqrt(2.0)

    C = np.linspace(-1.0, 1.0, F + 2, dtype=np.float64)[1:-1]
    C42 = float(C[F - 1])

    nc = bacc.Bacc("TRN2", target_bir_lowering=False, debug=False, num_devices=8)
    xyz_d = nc.dram_tensor("xyz", [P, D], DT, kind="ExternalInput").ap()
    nbr_d = nc.dram_tensor("nbr", [P, D], DT, kind="ExternalInput").ap()
    # blob: ident | whi' | wlo6' | negc2 | e3s | rmat3
    NB = 128 + OUT + OUT + 1 + 128 + 96
    blob_d = nc.dram_tensor("blob", [128, NB], DTR, kind="ExternalInput").ap()
    out_d = nc.dram_tensor("out", [P, OUT], DT, kind="ExternalOutput").ap()

    with TileContext(nc) as tc:
        with (
            tc.tile_pool(name="const", bufs=1) as constp,
            tc.tile_pool(name="stats", bufs=1) as statp,
            tc.tile_pool(name="gwork", bufs=8) as gwp,
            tc.tile_pool(name="flopool", bufs=1) as flop,
            tc.tile_pool(name="main", bufs=6) as mainp,
        ):
            ppsum = tc.alloc_tile_pool(name="ppsum", bufs=2, space="PSUM")

            # ---- input + constant loads (n first: stats start earliest) ----
            nbr_g = nbr_d.rearrange("(g k) d -> g (k d)", k=K)
            xyz_g = xyz_d.rearrange("(g k) d -> g (k d)", k=K)
            n_all = gwp.tile([128, NT * K * D], DT, tag="nall", bufs=1)
            nc.sync.dma_start(
                n_all.rearrange("p (t f) -> p t f", f=K * D),
                nbr_g.rearrange("(t p) f -> p t f", p=128),
            )
            x_all = gwp.tile([128, NT * K * D], DT, tag="xall", bufs=1)
            nc.scalar.dma_start(
                x_all.rearrange("p (t f) -> p t f", f=K * D),
                xyz_g.rearrange("(t p) f -> p t f", p=128),
            )
            blob = constp.tile([128, NB], DTR)
            nc.sync.dma_start(blob[:], blob_d[:])
            ident = blob[:, 0:128].bitcast(DT)
            whi = blob[:, 128 : 128 + OUT]
            wlo = blob[0:6, 128 + OUT : 128 + 2 * OUT]
            negc2 = blob[:, 128 + 2 * OUT : 128 + 2 * OUT + 1].bitcast(DT)
            e3s = blob[:, 128 + 2 * OUT + 1 : 128 + 2 * OUT + 1 + 128]
            rmat3 = blob[0:D, 128 + 2 * OUT + 1 + 128 : NB]
            cb42 = constp.tile([96, 1], DT)
            nc.vector.memset(cb42[:], -SQ2 * C42)

            # ---- raw second moments + mean (all DVE, no centering) ----
            # n free layout per group: (k, d) interleaved
            n_tdk = n_all.rearrange("p (t k d) -> p t d k", k=K, d=D)
            S_all = statp.tile([128, NT * 6], DT)
            S_v = S_all.rearrange("p (t i) -> p t i", i=6)
            # diag (0,0),(1,1),(2,2)
            prod3 = gwp.tile([128, NT * 3 * K], DT, tag="prod3", bufs=1)
            p3v = prod3.rearrange("p (t i k) -> p t i k", i=3, k=K)
            nc.vector.tensor_mul(p3v, n_tdk, n_tdk)
            nc.vector.tensor_reduce(S_v[:, :, 0:3], p3v, axis=X, op=Op.add)
            # off-diag (0,1),(1,2)
            prod2 = gwp.tile([128, NT * 2 * K], DT, tag="prod2", bufs=1)
            p2v = prod2.rearrange("p (t i k) -> p t i k", i=2, k=K)
            nc.vector.tensor_mul(p2v, n_tdk[:, :, 0:2, :], n_tdk[:, :, 1:3, :])
            nc.vector.tensor_reduce(S_v[:, :, 3:5], p2v, axis=X, op=Op.add)
            # off-diag (0,2)
            prod1 = gwp.tile([128, NT * K], DT, tag="prod1", bufs=1)
            p1v = prod1.rearrange("p (t k) -> p t () k", k=K)
            nc.vector.tensor_mul(
                p1v, n_tdk[:, :, 0:1, :], n_tdk[:, :, 2:3, :]
            )
            nc.vector.tensor_reduce(S_v[:, :, 5:6], p1v, axis=X, op=Op.add)
            # mean
            m_all = statp.tile([128, NT * 3], DT)
            m_v = m_all.rearrange("p (t d) -> p t d", d=D)
            nc.vector.tensor_reduce(m_v, n_tdk, axis=X, op=Op.add)
            nc.vector.tensor_scalar_mul(m_all[:], m_all[:], 1.0 / K)
            # U = S - K m_i m_j   (U = (K-1) * cov, unnormalized)
            mm_all = statp.tile([128, NT * 6], DT)
            mm_v = mm_all.rearrange("p (t i) -> p t i", i=6)
            nc.vector.tensor_mul(mm_v[:, :, 0:3], m_v, m_v)
            nc.vector.tensor_mul(mm_v[:, :, 3:5], m_v[:, :, 0:2], m_v[:, :, 1:3])
            nc.vector.tensor_mul(mm_v[:, :, 5:6], m_v[:, :, 0:1], m_v[:, :, 2:3])
            U_all = statp.tile([128, NT * 6], DT)
            U_v = U_all.rearrange("p (t i) -> p t i", i=6)
            nc.vector.scalar_tensor_tensor(
                U_v, mm_v, -float(K), S_v, op0=Op.mult, op1=Op.add
            )

            # ---- char-poly coefficients: det(U - xI) = -x^3 + c2 x^2 + c1 x + c0
            Ud = U_v[:, :, 0:3]
            Uo = U_v[:, :, 3:6]
            c2_t = statp.tile([128, NT], DT)
            c2v = c2_t.rearrange("p t -> p t ()")
            nc.vector.tensor_reduce(c2v, Ud, axis=X, op=Op.add)
            soff = statp.tile([128, NT * 3], DT)
            soffv = soff.rearrange("p (t i) -> p t i", i=3)
            nc.vector.tensor_mul(soffv, Uo, Uo)          # f2 g2 h2
            p1_t = statp.tile([128, NT], DT)
            p1vv = p1_t.rearrange("p t -> p t ()")
            nc.vector.tensor_reduce(p1vv, soffv, axis=X, op=Op.add)
            pd2 = statp.tile([128, NT * 2], DT)
            pd2v = pd2.rearrange("p (t i) -> p t i", i=2)
            nc.vector.tensor_mul(pd2v, Ud[:, :, 0:2], Ud[:, :, 1:3])  # ab bc
            pd1 = statp.tile([128, NT], DT)
            pd1v = pd1.rearrange("p t -> p t ()")
            nc.vector.tensor_mul(pd1v, Ud[:, :, 0:1], Ud[:, :, 2:3])  # ac
            Ps = statp.tile([128, NT], DT)
            Psv = Ps.rearrange("p t -> p t ()")
            nc.vector.tensor_reduce(Psv, pd2v, axis=X, op=Op.add)
            nc.vector.tensor_add(Ps[:], Ps[:], pd1[:])   # ab+bc+ac
            c1_t = statp.tile([128, NT], DT)
            nc.vector.tensor_sub(c1_t[:], p1_t[:], Ps[:])  # f2+g2+h2 - (ab+bc+ac)
            # c0 = det = abc + 2fgh - a g2 - b h2 - c f2
            abc = statp.tile([128, NT], DT)
            abcv = abc.rearrange("p t -> p t ()")
            nc.vector.tensor_mul(abcv, pd2v[:, :, 0:1], Ud[:, :, 2:3])
            fgh = statp.tile([128, NT], DT)
            fghv = fgh.rearrange("p t -> p t ()")
            nc.vector.tensor_mul(fghv, Uo[:, :, 0:1], Uo[:, :, 1:2])
            nc.vector.tensor_mul(fghv, fghv, Uo[:, :, 2:3])
            mix2 = statp.tile([128, NT * 2], DT)
            mix2v = mix2.rearrange("p (t i) -> p t i", i=2)
            nc.vector.tensor_mul(mix2v, Ud[:, :, 0:2], soffv[:, :, 1:3])  # a g2, b h2
            mix1 = statp.tile([128, NT], DT)
            mix1v = mix1.rearrange("p t -> p t ()")
            nc.vector.tensor_mul(mix1v, Ud[:, :, 2:3], soffv[:, :, 0:1])  # c f2
            mixs = statp.tile([128, NT], DT)
            mixsv = mixs.rearrange("p t -> p t ()")
            nc.vector.tensor_reduce(mixsv, mix2v, axis=X, op=Op.add)
            nc.vector.tensor_add(mixs[:], mixs[:], mix1[:])
            c0_t = statp.tile([128, NT], DT)
            nc.vector.scalar_tensor_tensor(
                c0_t[:], fgh[:], 2.0, abc[:], op0=Op.mult, op1=Op.add
            )
            nc.vector.tensor_sub(c0_t[:], c0_t[:], mixs[:])

            # ---- Newton from x=0 (monotone from below for PSD) ----
            xe = statp.tile([128, NT], DT)
            nc.vector.memset(xe[:], 0.0)
            t1 = statp.tile([128, NT], DT)
            t2 = statp.tile([128, NT], DT)
            pe_ = statp.tile([128, NT], DT)
            x2 = statp.tile([128, NT], DT)
            dpe = statp.tile([128, NT], DT)
            rec = statp.tile([128, NT], DT)
            for it in range(6):
                nc.vector.tensor_sub(t1[:], c2_t[:], xe[:])          # c2 - x
                nc.vector.tensor_mul(t2[:], t1[:], xe[:])            # c2 x - x^2
                nc.vector.tensor_add(pe_[:], t2[:], c1_t[:])         # + c1
                nc.vector.tensor_mul(pe_[:], pe_[:], xe[:])          # * x
                nc.vector.tensor_add(pe_[:], pe_[:], c0_t[:])        # p(x)
                nc.vector.tensor_mul(x2[:], xe[:], xe[:])
                nc.vector.scalar_tensor_tensor(
                    dpe[:], t2[:], 2.0, x2[:], op0=Op.mult, op1=Op.subtract
                )                                                    # 2t2 - x^2
                nc.vector.tensor_add(dpe[:], dpe[:], c1_t[:])        # p'(x)
                nc.vector.reciprocal(rec[:], dpe[:])
                nc.vector.tensor_mul(rec[:], pe_[:], rec[:])         # p/p'
                nc.vector.tensor_sub(xe[:], xe[:], rec[:])
            den = statp.tile([128, NT], DT)
            nc.vector.tensor_scalar_add(den[:], c2_t[:], (K - 1) * 1e-6)
            nc.vector.reciprocal(den[:], den[:])
            curv_all = statp.tile([128, NT], DT)
            nc.vector.tensor_mul(curv_all[:], xe[:], den[:])

            # curv -> [1, G] natural order
            cps = ppsum.tile([8, 128], DT, tag="pp")
            nc.tensor.transpose(cps[:], curv_all[:], ident)
            ctv = statp.tile([8, 128], DT)
            nc.vector.tensor_copy(ctv[:], cps[:])
            curv_g = statp.tile([1, G], DT)
            nc.scalar.dma_start(
                curv_g.rearrange("o (t g) -> o t g", g=128), ctv[:]
            )

            # ---- kd-major transpose: xdkT row 3k+d, col g ----
            xdkT = statp.tile([96, G], DT)
            for t in range(NT):
                xps = ppsum.tile([96, 128], DT, tag="tp")
                nc.tensor.transpose(
                    xps[:], x_all[:, t * K * D : (t + 1) * K * D], ident
                )
                nc.scalar.copy(xdkT[:, t * 128 : (t + 1) * 128], xps[:])

            # x3s: quadrant-aligned per-k slabs [32q+d, (k%11)*G + g], q=k//11
            # (matmul operand bases must be 0/32/64, so 3 quadrants of <=11)
            x3s = statp.tile([96, 11 * G], DTR)
            xdkT_kd = xdkT.rearrange("(k d) g -> k d g", d=D)
            for q in range(3):
                cnt = 11 if q < 2 else 10
                for d in range(D):
                    nc.scalar.dma_start(
                        x3s[
                            32 * q + d : 32 * q + d + 1, 0 : cnt * G
                        ].rearrange("o (k g) -> o k g", g=G),
                        xdkT_kd[11 * q : 11 * q + cnt, d : d + 1, :]
                        .squeeze(1)
                        .bitcast(DTR),
                    )

            # mean rows m3 [3, G] (natural col order) via per-d transposes
            m_td = m_all.rearrange("p (t d) -> p d t", d=D)
            mdt = statp.tile([96, 128], DT)
            for d in range(D):
                mps = ppsum.tile([8, 128], DT, tag="pp")
                nc.tensor.transpose(mps[:], m_td[:, d : d + 1, :].squeeze(1), ident)
                nc.vector.tensor_copy(mdt[d * K : d * K + NT, :], mps[:])
            m3 = statp.tile([D, G], DTR)
            for d in range(D):
                nc.scalar.dma_start(
                    m3[d : d + 1, :].rearrange("o (t g) -> o t g", g=128),
                    mdt[d * K : d * K + NT, :].bitcast(DTR),
                )

            # lap rows: |xdkT - mean|, kd-major; rmat3[d, 3k+d'] = [d==d']
            lapT = statp.tile([96, G], DT)
            for half in range(2):
                sl = slice(half * 512, (half + 1) * 512)
                mrep = ppsum.tile([96, 512], DT, tag="mr")
                nc.tensor.matmul(
                    mrep[:], rmat3[:], m3[:, sl], start=True, stop=True
                )
                nc.vector.tensor_sub(lapT[:, sl], xdkT[:, sl], mrep[:])
            nc.scalar.activation(lapT[:], lapT[:], Act.Abs)

            # g42 row source: DErf over all 96 rows (only d=2 rows consumed)
            g42f = statp.tile([96, G], DT)
            nc.scalar.activation(
                g42f[:], xdkT[:], Act.Derivative_Erf, bias=cb42[:], scale=SQ2
            )

            # ones row source (folds projection bias into the matmul)
            HK = K // 4
            ones_t = gwp.tile([128, HK * G // 128], DT, tag="ones", bufs=1)
            nc.vector.memset(ones_t[:], 1.0)

            lap_kd = lapT.rearrange("(k d) g -> k d g", d=D)
            g42_kd = g42f.rearrange("(k d) g -> k d g", d=D)

            ppsum.release()
            xbp = tc.alloc_tile_pool(name="xbpsum", bufs=2, space="PSUM")
            outp = tc.alloc_tile_pool(name="outpsum", bufs=3, space="PSUM")

            # ---- main loop: 4 phases x 8 k x 2 halves of 512 points ----
            for phase in range(4):
                k0 = phase * HK
                flo = flop.tile([6, HK * G], DTR, tag="flo", bufs=2)
                nc.scalar.dma_start(
                    flo[0:1, :].rearrange("o (k g) -> o k g", g=G),
                    g42_kd[k0 : k0 + HK, 2:3, :].squeeze(1).bitcast(DTR),
                )
                nc.scalar.dma_start(
                    flo[1:2, :].rearrange("o (k g) -> o k g", g=G),
                    curv_g.bitcast(DTR).unsqueeze(1).broadcast_to([1, HK, G]),
                )
                for d in range(D):
                    nc.scalar.dma_start(
                        flo[2 + d : 3 + d, :].rearrange("o (k g) -> o k g", g=G),
                        lap_kd[k0 : k0 + HK, d : d + 1, :].squeeze(1).bitcast(DTR),
                    )
                nc.scalar.dma_start(
                    flo[5:6, :].rearrange("o (a b) -> o a b", b=HK * G // 128),
                    ones_t.bitcast(DTR),
                )
                for k in range(k0, k0 + HK):
                    q, j = k // 11, k % 11
                    for half in range(2):
                        csl = slice(j * G + half * 512, j * G + (half + 1) * 512)
                        xb = xbp.tile([128, 512], DT, tag="xb")
                        nc.tensor.matmul(
                            xb[:],
                            e3s[32 * q : 32 * q + 3, :],
                            x3s[32 * q : 32 * q + 3, csl],
                            start=True,
                            stop=True,
                        )
                        fhi = mainp.tile([128, 512], DTR, tag="fhi")
                        nc.scalar.activation(
                            fhi[:], xb[:], Act.Derivative_Erf, bias=negc2, scale=SQ2
                        )
                        so = mainp.tile([128, 4 * OUT], DT, tag="so", bufs=6)
                        for pair in range(2):
                            ps = outp.tile([128, 1024], DT, tag="ps")
                            for c in range(2):
                                jj = pair * 2 + c
                                nc.tensor.matmul(
                                    ps[:, c * 512 : c * 512 + OUT],
                                    fhi[:, jj * 128 : (jj + 1) * 128],
                                    whi,
                                    start=True,
                                    stop=False,
                                )
                                lo = (k - k0) * G + half * 512 + jj * 128
                                nc.tensor.matmul(
                                    ps[:, c * 512 : c * 512 + OUT],
                                    flo[0:6, lo : lo + 128],
                                    wlo,
                                    start=False,
                                    stop=True,
                                )
                            ps_v = ps.rearrange("p (c x) -> p c x", x=512)[:, :, 0:OUT]
                            so_v = so.rearrange("p (c x) -> p c x", x=OUT)[
                                :, pair * 2 : pair * 2 + 2, :
                            ]
                            cnt = (k * 2 + half) * 2 + pair
                            if cnt % 2 == 1:
                                nc.scalar.copy(so_v, ps_v)
                            else:
                                nc.vector.tensor_copy(so_v, ps_v)
                        out_ap = (
                            out_d.rearrange("(g k) x -> k g x", k=K)[
                                k : k + 1, half * 512 : (half + 1) * 512, :
                            ]
                            .squeeze(0)
                            .rearrange("(c g) x -> g c x", c=4)
                        )
                        nc.sync.dma_start(
                            out_ap, so.rearrange("p (c x) -> p c x", x=OUT)
                        )
            outp.release()
            xbp.release()

    nc.compile()
    return nc


def _get_program():
    if "nc" not in _prog_cache:
        _prog_cache["nc"] = _build_program()
    return _prog_cache["nc"]


def kernel(xyz, neighbor_xyz, projection_weight, projection_bias):
    from concourse.bass_utils import run_bass_kernel_spmd

    nc = _get_program()

    w = np.ascontiguousarray(projection_weight, dtype=np.float32)
    bias = np.ascontiguousarray(projection_bias, dtype=np.float32)
    derf_fold = math.sqrt(math.pi) / 2.0
    whi = np.ascontiguousarray(w[:128]) * derf_fold
    wlo6 = np.concatenate(
        [w[128:129] * derf_fold, w[129:TOTAL], bias[None, :]], axis=0
    ).astype(np.float32)

    ident = np.eye(128, dtype=np.float32)
    c = np.linspace(-1.0, 1.0, F + 2, dtype=np.float32)[1:-1]
    cr = c[np.arange(128) % F]
    negc2 = (-math.sqrt(2.0) * cr).reshape(128, 1).astype(np.float32)
    e3s = np.zeros((128, 128), dtype=np.float32)
    for q in range(4):
        for r in range(128):
            e3s[32 * q + min(r // F, 2), r] = 1.0

    rmat3 = np.zeros((D, 96), dtype=np.float32)
    for d in range(D):
        rmat3[d, np.arange(K) * D + d] = 1.0

    NB = 128 + OUT + OUT + 1 + 128 + 96
    blob = np.zeros((128, NB), dtype=np.float32)
    blob[:, 0:128] = ident
    blob[:, 128 : 128 + OUT] = whi
    blob[0:6, 128 + OUT : 128 + 2 * OUT] = wlo6
    blob[:, 128 + 2 * OUT : 128 + 2 * OUT + 1] = negc2
    blob[:, 128 + 2 * OUT + 1 : 128 + 2 * OUT + 1 + 128] = e3s
    blob[0:D, 128 + 2 * OUT + 1 + 128 : NB] = rmat3

    xyz = np.ascontiguousarray(xyz, dtype=np.float32)
    nbr = np.ascontiguousarray(neighbor_xyz, dtype=np.float32)
    in_maps = []
    for core in range(B):
        in_maps.append(
            {
                "xyz": xyz[core].reshape(P, D),
                "nbr": nbr[core].reshape(P, D),
                "blob": blob,
            }
        )
    res = run_bass_kernel_spmd(nc, in_maps, list(range(B)))
    out = np.stack(
        [res.results[i]["out"].reshape(S, K, OUT) for i in range(B)], axis=0
    )
    return out
